# revision 8
# baseline (speedup 1.0000x reference)
"""Trainium2 Bass kernel: K-step Euler rollout of a kinematic bicycle model.

Full inputs:
  initial_state [131072, 4] f32, controls [131072, 64, 2] f32,
  timestep scalar f32, agents_pars [131072, 2] f32
Output: [131072, 64, 4] f32 (state after each of the 64 steps).

Strategy: pure data parallel over 8 NeuronCores (16384 agents each).
Per core the sequential rollout is decomposed into 4 segmented prefix
scans (DVE tensor_tensor_scan with a 0/1 mask resetting state at agent
boundaries):
    V    = dt*vel   : V_k = V_{k-1} + dt^2*accel_k        (65-slot, slot0 = dt*vel0)
    yaw  : yaw_k = yaw_{k-1} + V_ex_k*tan(steer_k)/L      (65-slot, slot0 = yaw0)
    x    : x_k = x_{k-1} + V_ex_k*cos(yaw_ex_k)           (writes output lane)
    y    : y_k = y_{k-1} + V_ex_k*sin(yaw_ex_k)           (writes output lane)
Trig on ScalarE (Sin table; cos(yaw) via 1-2*sin^2(yaw/2) to stay in
[-pi,pi]); tensor products on GPSIMD; reciprocal on DVE.
"""
import os
import sys

for _p in ("/opt/trn_rl_repo", "/root/.axon_site/_ro/trn_rl_repo"):
    if os.path.isdir(_p) and _p not in sys.path:
        sys.path.insert(0, _p)

import numpy as np
import concourse.bass as bass
import concourse.bacc as bacc
import concourse.tile as tile
from concourse import mybir

F32 = mybir.dt.float32
AF = mybir.ActivationFunctionType
ALU = mybir.AluOpType

B = 131072
K = 64
NCORES = 8
BC = B // NCORES          # 16384 agents per core
P = 128                   # partitions
AG = 16                   # agents per partition per group
GRP = BC // (P * AG)      # 4 groups per core
PI = float(np.pi)

_cache = {}


FLAGS = set(os.environ.get("KFLAGS", "").split(","))


def _build(dt: float):
    """Build the per-core SPMD program (identical on all 8 cores)."""
    nc = bacc.Bacc("TRN2", debug=False)

    d_init = nc.dram_tensor("initial_state", [BC, 4], F32, kind="ExternalInput").ap()
    d_ctrl = nc.dram_tensor("controls", [BC, K, 2], F32, kind="ExternalInput").ap()
    d_pars = nc.dram_tensor("agents_pars", [BC, 2], F32, kind="ExternalInput").ap()
    d_out = nc.dram_tensor("out", [BC, K, 4], F32, kind="ExternalOutput").ap()

    r_init = d_init.rearrange("(g p a) c -> g p (a c)", g=GRP, p=P, a=AG)
    r_ctrl = d_ctrl.rearrange("(g p a) k c -> g p (a k c)", g=GRP, p=P, a=AG)
    r_pars = d_pars.rearrange("(g p a) c -> g p (a c)", g=GRP, p=P, a=AG)
    r_out = d_out.rearrange("(g p a) k c -> g p (a k c)", g=GRP, p=P, a=AG)

    flat = lambda t: t.rearrange("p a k -> p (a k)")

    with tile.TileContext(nc) as tc:
        with (
            tc.tile_pool(name="consts", bufs=1) as consts,
            tc.tile_pool(name="io", bufs=2) as io,
            tc.tile_pool(name="mid", bufs=1) as mid,
        ):
            # constants
            BF16 = mybir.dt.bfloat16
            mask65 = consts.tile([P, AG, 65], BF16)
            nc.vector.memset(mask65, 1.0)
            nc.vector.memset(mask65[:, :, 0], 0.0)
            mask64 = consts.tile([P, AG, 64], BF16)
            nc.vector.memset(mask64, 1.0)
            nc.vector.memset(mask64[:, :, 0], 0.0)
            c_dt2 = consts.tile([P, 1], F32)
            nc.vector.memset(c_dt2, dt * dt)
            c_pi2 = consts.tile([P, 1], F32)
            nc.vector.memset(c_pi2, PI / 2)
            c_m1 = consts.tile([P, 1], F32)
            nc.vector.memset(c_m1, -1.0)
            c_dt = consts.tile([P, 1], F32)
            nc.vector.memset(c_dt, dt)
            c_invdt = consts.tile([P, 1], F32)
            nc.vector.memset(c_invdt, 1.0 / dt)

            for g in range(GRP):
                ctrl_t = io.tile([P, AG, K, 2], F32, tag="ctrl", bufs=3)
                init_t = io.tile([P, AG, 4], F32, tag="init", bufs=3)
                pars_t = io.tile([P, AG, 2], F32, tag="pars", bufs=3)
                out_t = io.tile([P, AG, K, 4], F32, tag="out", bufs=3)
                nc.sync.dma_start(ctrl_t, r_ctrl[g])
                nc.sync.dma_start(init_t, r_init[g])
                nc.sync.dma_start(pars_t, r_pars[g])

                x0 = init_t[:, :, 0]
                y0 = init_t[:, :, 1]
                yaw0 = init_t[:, :, 2]
                vel0 = init_t[:, :, 3]
                accel = ctrl_t[:, :, :, 0]
                steer = ctrl_t[:, :, :, 1]

                invL = mid.tile([P, AG], F32)
                nc.vector.reciprocal(invL, pars_t[:, :, 0])

                # ---- velocity chain (V = dt*vel), 65-slot ----
                t165 = mid.tile([P, AG, 65], F32, tag="t165", bufs=3)
                nc.scalar.activation(t165[:, :, 1:65], accel, AF.Copy, scale=c_dt2)
                nc.scalar.activation(t165[:, :, 0], vel0, AF.Copy, scale=c_dt)
                V65 = t165
                nc.vector.tensor_tensor_scan(
                    flat(V65), flat(mask65), flat(t165), 0.0, ALU.mult, ALU.add
                )
                Vex = V65[:, :, 0:64]     # dt * vel before each step
                Vincl = V65[:, :, 1:65]   # dt * vel after each step

                # ---- steering: tan = sin * recip(cos) ----
                sinS = mid.tile([P, AG, K], F32, tag="sinS", bufs=2)
                nc.scalar.activation(sinS, steer, AF.Sin)
                cosS = mid.tile([P, AG, K], F32, tag="cosS", bufs=2)
                nc.scalar.activation(cosS, steer, AF.Sin, bias=c_pi2)
                recipC = mid.tile([P, AG, K], F32, tag="recipC", bufs=2)
                nc.vector.reciprocal_approx_fast(flat(recipC), flat(cosS))
                if "noinplace" in FLAGS:
                    tanS = mid.tile([P, AG, K], F32, tag="w265x", name="tanS")
                else:
                    tanS = sinS
                nc.gpsimd.tensor_mul(tanS, sinS, recipC)
                tanL = mid.tile([P, AG, K], F32, tag="tanL", bufs=2)
                nc.gpsimd.tensor_mul(
                    tanL, tanS, invL.unsqueeze(2).broadcast_to([P, AG, K])
                )

                # ---- yaw chain, 65-slot ----
                w265 = mid.tile([P, AG, 65], F32, tag="w265", bufs=3)
                nc.gpsimd.tensor_mul(w265[:, :, 1:65], Vex, tanL)
                nc.scalar.activation(w265[:, :, 0], yaw0, AF.Copy)
                Y65 = w265
                nc.vector.tensor_tensor_scan(
                    flat(Y65), flat(mask65), flat(w265), 0.0, ALU.mult, ALU.add
                )
                Yex = Y65[:, :, 0:64]
                Yincl = Y65[:, :, 1:65]

                # ---- trig of yaw: sin direct; cos = Sin(pi/2 - |yaw|) ----
                sinY = mid.tile([P, AG, K], F32, tag="sinY", bufs=2)
                nc.scalar.activation(sinY, Yex, AF.Sin)
                yin = mid.tile([P, AG, K], F32, tag="yin", bufs=2)
                nc.gpsimd.tensor_mul(yin, Vex, sinY)
                nc.vector.tensor_add(yin[:, :, 0], yin[:, :, 0], y0)
                ylane = out_t[:, :, :, 1].rearrange("p a k -> p (a k)")
                nc.vector.tensor_tensor_scan(
                    ylane, flat(mask64), flat(yin), 0.0, ALU.mult, ALU.add
                )

                absY = mid.tile([P, AG, K], F32, tag="absY", bufs=2)
                nc.scalar.activation(absY, Yex, AF.Abs)
                cosY = mid.tile([P, AG, K], F32, tag="cosY", bufs=2)
                nc.scalar.activation(cosY, absY, AF.Sin, scale=c_m1, bias=c_pi2)
                xin = mid.tile([P, AG, K], F32, tag="xin", bufs=2)
                nc.gpsimd.tensor_mul(xin, Vex, cosY)
                nc.vector.tensor_add(xin[:, :, 0], xin[:, :, 0], x0)
                xlane = out_t[:, :, :, 0].rearrange("p a k -> p (a k)")
                nc.vector.tensor_tensor_scan(
                    xlane, flat(mask64), flat(xin), 0.0, ALU.mult, ALU.add
                )

                # ---- vel / yaw output lanes ----
                nc.scalar.activation(out_t[:, :, :, 3], Vincl, AF.Copy, scale=c_invdt)
                if "yawcopy" in FLAGS:
                    nc.vector.tensor_copy(out_t[:, :, :, 2], Yincl)
                else:
                    nc.vector.tensor_scalar(out_t[:, :, :, 2], Yincl, 1.0, None, ALU.mult)

                nc.sync.dma_start(r_out[g], out_t.rearrange("p a k c -> p (a k c)"))

    nc.compile()
    return nc


def _get(dt: float):
    key = round(float(dt), 12)
    if key not in _cache:
        _cache[key] = _build(float(dt))
    return _cache[key]


def kernel(initial_state, controls, timestep, agents_pars, _trace=False):
    initial_state = np.ascontiguousarray(np.asarray(initial_state, dtype=np.float32))
    controls = np.ascontiguousarray(np.asarray(controls, dtype=np.float32))
    agents_pars = np.ascontiguousarray(np.asarray(agents_pars, dtype=np.float32))
    dt = float(np.asarray(timestep, dtype=np.float32))

    nc = _get(dt)
    in_maps = []
    for c in range(NCORES):
        s = slice(c * BC, (c + 1) * BC)
        in_maps.append(
            {
                "initial_state": initial_state[s],
                "controls": controls[s],
                "agents_pars": agents_pars[s],
            }
        )
    from concourse import bass_utils

    r = bass_utils.run_bass_kernel_spmd(
        nc, in_maps, core_ids=list(range(NCORES)), trace=_trace
    )
    out = np.concatenate([r.results[c]["out"] for c in range(NCORES)], axis=0)
    if _trace:
        kernel.last_result = r
    return out


if __name__ == "__main__":
    # quick CoreSim check on one core's shard
    from concourse.bass_interp import CoreSim

    rng = np.random.default_rng(0)
    init = np.stack(
        [
            rng.normal(0, 10, BC),
            rng.normal(0, 10, BC),
            rng.normal(0, 0.5, BC),
            rng.normal(5, 2, BC),
        ],
        axis=-1,
    ).astype(np.float32)
    ctrl = (rng.standard_normal((BC, K, 2)) * np.array([1.0, 0.05])).astype(np.float32)
    pars = np.stack(
        [3 + 3 * rng.random(BC), 1.5 + rng.random(BC)], axis=-1
    ).astype(np.float32)
    dt = np.float32(0.1)

    nc = _get(float(dt))
    sim = CoreSim(nc, trace=False)
    sim.tensor("initial_state")[:] = init
    sim.tensor("controls")[:] = ctrl
    sim.tensor("agents_pars")[:] = pars
    sim.simulate(check_with_hw=False)
    got = np.array(sim.tensor("out"))

    # numpy reference
    x, y, yaw, vel = (init[:, i].astype(np.float64) for i in range(4))
    L = pars[:, 0].astype(np.float64)
    exp = np.zeros((BC, K, 4))
    dtf = float(dt)
    for k in range(K):
        a = ctrl[:, k, 0].astype(np.float64)
        s = ctrl[:, k, 1].astype(np.float64)
        x = x + dtf * vel * np.cos(yaw)
        y = y + dtf * vel * np.sin(yaw)
        yaw = yaw + dtf * vel * np.tan(s) / L
        vel = vel + dtf * a
        exp[:, k] = np.stack([x, y, yaw, vel], axis=-1)
    err = np.linalg.norm(got - exp) / np.linalg.norm(exp)
    print("CoreSim relnorm vs numpy ref:", err)
    for c in range(4):
        e = np.abs(got[:, :, c] - exp[:, :, c]).max()
        print(f"  lane {c}: absmax {e:.3e}")


# revision 10
# speedup vs baseline: 1.0171x; 1.0171x over previous
"""Trainium2 Bass kernel: K-step Euler rollout of a kinematic bicycle model.

Full inputs:
  initial_state [131072, 4] f32, controls [131072, 64, 2] f32,
  timestep scalar f32, agents_pars [131072, 2] f32
Output: [131072, 64, 4] f32 (state after each of the 64 steps).

Strategy: pure data parallel over 8 NeuronCores (16384 agents each).
Per core the sequential rollout is decomposed into 4 segmented prefix
scans (DVE tensor_tensor_scan with a 0/1 mask resetting state at agent
boundaries):
    V    = dt*vel   : V_k = V_{k-1} + dt^2*accel_k        (65-slot, slot0 = dt*vel0)
    yaw  : yaw_k = yaw_{k-1} + V_ex_k*tan(steer_k)/L      (65-slot, slot0 = yaw0)
    x    : x_k = x_{k-1} + V_ex_k*cos(yaw_ex_k)           (writes output lane)
    y    : y_k = y_{k-1} + V_ex_k*sin(yaw_ex_k)           (writes output lane)
Trig on ScalarE (Sin table; cos(yaw) via 1-2*sin^2(yaw/2) to stay in
[-pi,pi]); tensor products on GPSIMD; reciprocal on DVE.
"""
import os
import sys

for _p in ("/opt/trn_rl_repo", "/root/.axon_site/_ro/trn_rl_repo"):
    if os.path.isdir(_p) and _p not in sys.path:
        sys.path.insert(0, _p)

import numpy as np
import concourse.bass as bass
import concourse.bacc as bacc
import concourse.tile as tile
from concourse import mybir

F32 = mybir.dt.float32
AF = mybir.ActivationFunctionType
ALU = mybir.AluOpType

B = 131072
K = 64
NCORES = 8
BC = B // NCORES          # 16384 agents per core
P = 128                   # partitions
AG = 16                   # agents per partition per group
GRP = BC // (P * AG)      # 4 groups per core
PI = float(np.pi)

_cache = {}


FLAGS = set(os.environ.get("KFLAGS", "").split(","))


def _build(dt: float):
    """Build the per-core SPMD program (identical on all 8 cores)."""
    nc = bacc.Bacc("TRN2", debug=False)

    d_aux = nc.dram_tensor("aux", [BC, 6], F32, kind="ExternalInput").ap()
    d_ctrl = nc.dram_tensor("controls", [BC, K, 2], F32, kind="ExternalInput").ap()
    d_out = nc.dram_tensor("out", [BC, K, 4], F32, kind="ExternalOutput").ap()

    r_aux = d_aux.rearrange("(g p a) c -> g p (a c)", g=GRP, p=P, a=AG)
    r_ctrl = d_ctrl.rearrange("(g p a) k c -> g p (a k c)", g=GRP, p=P, a=AG)
    r_out = d_out.rearrange("(g p a) k c -> g p (a k c)", g=GRP, p=P, a=AG)

    flat = lambda t: t.rearrange("p a k -> p (a k)")

    with tile.TileContext(nc) as tc:
        with (
            tc.tile_pool(name="consts", bufs=1) as consts,
            tc.tile_pool(name="io", bufs=2) as io,
            tc.tile_pool(name="mid", bufs=1) as mid,
        ):
            # constants
            BF16 = mybir.dt.bfloat16
            mask65 = consts.tile([P, AG, 65], BF16)
            nc.vector.memset(mask65, 1.0)
            nc.vector.memset(mask65[:, :, 0], 0.0)
            mask64 = consts.tile([P, AG, 64], BF16)
            nc.vector.memset(mask64, 1.0)
            nc.vector.memset(mask64[:, :, 0], 0.0)
            c_dt2 = consts.tile([P, 1], F32)
            nc.vector.memset(c_dt2, dt * dt)
            c_pi2 = consts.tile([P, 1], F32)
            nc.vector.memset(c_pi2, PI / 2)
            c_m1 = consts.tile([P, 1], F32)
            nc.vector.memset(c_m1, -1.0)
            c_dt = consts.tile([P, 1], F32)
            nc.vector.memset(c_dt, dt)
            c_invdt = consts.tile([P, 1], F32)
            nc.vector.memset(c_invdt, 1.0 / dt)

            for g in range(GRP):
                ctrl_t = io.tile([P, AG, K, 2], F32, tag="ctrl", bufs=3)
                aux_t = io.tile([P, AG, 6], F32, tag="aux", bufs=3)
                out_t = io.tile([P, AG, K, 4], F32, tag="out", bufs=3)
                nc.scalar.dma_start(ctrl_t, r_ctrl[g])
                nc.scalar.dma_start(aux_t, r_aux[g])
                init_t = aux_t  # views below
                pars_t = aux_t[:, :, 4:6]

                x0 = init_t[:, :, 0]
                y0 = init_t[:, :, 1]
                yaw0 = init_t[:, :, 2]
                vel0 = init_t[:, :, 3]
                accel = ctrl_t[:, :, :, 0]
                steer = ctrl_t[:, :, :, 1]

                invL = mid.tile([P, AG], F32)
                nc.vector.reciprocal(invL, aux_t[:, :, 4])

                # ---- velocity chain (V = dt*vel), 65-slot ----
                t165 = mid.tile([P, AG, 65], F32, tag="t165", bufs=3)
                nc.scalar.activation(t165[:, :, 1:65], accel, AF.Copy, scale=c_dt2)
                nc.scalar.activation(t165[:, :, 0], vel0, AF.Copy, scale=c_dt)
                V65 = t165
                nc.vector.tensor_tensor_scan(
                    flat(V65), flat(mask65), flat(t165), 0.0, ALU.mult, ALU.add
                )
                Vex = V65[:, :, 0:64]     # dt * vel before each step
                Vincl = V65[:, :, 1:65]   # dt * vel after each step

                # ---- steering: tan = sin * recip(cos) ----
                sinS = mid.tile([P, AG, K], F32, tag="sinS", bufs=2)
                nc.scalar.activation(sinS, steer, AF.Sin)
                cosS = mid.tile([P, AG, K], F32, tag="cosS", bufs=2)
                nc.scalar.activation(cosS, steer, AF.Sin, bias=c_pi2)
                recipC = mid.tile([P, AG, K], F32, tag="recipC", bufs=2)
                nc.vector.reciprocal_approx_fast(flat(recipC), flat(cosS))
                if "noinplace" in FLAGS:
                    tanS = mid.tile([P, AG, K], F32, tag="w265x", name="tanS")
                else:
                    tanS = sinS
                nc.gpsimd.tensor_mul(tanS, sinS, recipC)
                tanL = mid.tile([P, AG, K], F32, tag="tanL", bufs=2)
                nc.gpsimd.tensor_mul(
                    tanL, tanS, invL.unsqueeze(2).broadcast_to([P, AG, K])
                )

                # ---- yaw chain, 65-slot ----
                w265 = mid.tile([P, AG, 65], F32, tag="w265", bufs=3)
                nc.gpsimd.tensor_mul(w265[:, :, 1:65], Vex, tanL)
                nc.scalar.activation(w265[:, :, 0], yaw0, AF.Copy)
                Y65 = w265
                nc.vector.tensor_tensor_scan(
                    flat(Y65), flat(mask65), flat(w265), 0.0, ALU.mult, ALU.add
                )
                Yex = Y65[:, :, 0:64]
                Yincl = Y65[:, :, 1:65]

                # ---- trig of yaw: sin direct; cos = Sin(pi/2 - |yaw|) ----
                sinY = mid.tile([P, AG, K], F32, tag="sinY", bufs=2)
                nc.scalar.activation(sinY, Yex, AF.Sin)
                yin = mid.tile([P, AG, K], F32, tag="yin", bufs=2)
                nc.gpsimd.tensor_mul(yin, Vex, sinY)
                nc.vector.tensor_add(yin[:, :, 0], yin[:, :, 0], y0)
                ylane = out_t[:, :, :, 1].rearrange("p a k -> p (a k)")
                nc.vector.tensor_tensor_scan(
                    ylane, flat(mask64), flat(yin), 0.0, ALU.mult, ALU.add
                )

                absY = mid.tile([P, AG, K], F32, tag="absY", bufs=2)
                nc.scalar.activation(absY, Yex, AF.Abs)
                cosY = mid.tile([P, AG, K], F32, tag="cosY", bufs=2)
                nc.scalar.activation(cosY, absY, AF.Sin, scale=c_m1, bias=c_pi2)
                xin = mid.tile([P, AG, K], F32, tag="xin", bufs=2)
                nc.gpsimd.tensor_mul(xin, Vex, cosY)
                nc.vector.tensor_add(xin[:, :, 0], xin[:, :, 0], x0)
                xlane = out_t[:, :, :, 0].rearrange("p a k -> p (a k)")
                nc.vector.tensor_tensor_scan(
                    xlane, flat(mask64), flat(xin), 0.0, ALU.mult, ALU.add
                )

                # ---- vel / yaw output lanes ----
                nc.scalar.activation(out_t[:, :, :, 3], Vincl, AF.Copy, scale=c_invdt)
                if "yawcopy" in FLAGS:
                    nc.vector.tensor_copy(out_t[:, :, :, 2], Yincl)
                else:
                    nc.vector.tensor_scalar(out_t[:, :, :, 2], Yincl, 1.0, None, ALU.mult)

                nc.sync.dma_start(r_out[g], out_t.rearrange("p a k c -> p (a k c)"))

    nc.compile()
    return nc


def _get(dt: float):
    key = round(float(dt), 12)
    if key not in _cache:
        _cache[key] = _build(float(dt))
    return _cache[key]


def kernel(initial_state, controls, timestep, agents_pars, _trace=False):
    initial_state = np.ascontiguousarray(np.asarray(initial_state, dtype=np.float32))
    controls = np.ascontiguousarray(np.asarray(controls, dtype=np.float32))
    agents_pars = np.ascontiguousarray(np.asarray(agents_pars, dtype=np.float32))
    dt = float(np.asarray(timestep, dtype=np.float32))

    nc = _get(dt)
    aux = np.concatenate([initial_state, agents_pars], axis=1)
    in_maps = []
    for c in range(NCORES):
        s = slice(c * BC, (c + 1) * BC)
        in_maps.append({"aux": aux[s], "controls": controls[s]})
    from concourse import bass_utils

    r = bass_utils.run_bass_kernel_spmd(
        nc, in_maps, core_ids=list(range(NCORES)), trace=_trace
    )
    out = np.concatenate([r.results[c]["out"] for c in range(NCORES)], axis=0)
    if _trace:
        kernel.last_result = r
    return out


if __name__ == "__main__":
    # quick CoreSim check on one core's shard
    from concourse.bass_interp import CoreSim

    rng = np.random.default_rng(0)
    init = np.stack(
        [
            rng.normal(0, 10, BC),
            rng.normal(0, 10, BC),
            rng.normal(0, 0.5, BC),
            rng.normal(5, 2, BC),
        ],
        axis=-1,
    ).astype(np.float32)
    ctrl = (rng.standard_normal((BC, K, 2)) * np.array([1.0, 0.05])).astype(np.float32)
    pars = np.stack(
        [3 + 3 * rng.random(BC), 1.5 + rng.random(BC)], axis=-1
    ).astype(np.float32)
    dt = np.float32(0.1)

    nc = _get(float(dt))
    sim = CoreSim(nc, trace=False)
    sim.tensor("aux")[:] = np.concatenate([init, pars], axis=1)
    sim.tensor("controls")[:] = ctrl
    sim.simulate(check_with_hw=False)
    got = np.array(sim.tensor("out"))

    # numpy reference
    x, y, yaw, vel = (init[:, i].astype(np.float64) for i in range(4))
    L = pars[:, 0].astype(np.float64)
    exp = np.zeros((BC, K, 4))
    dtf = float(dt)
    for k in range(K):
        a = ctrl[:, k, 0].astype(np.float64)
        s = ctrl[:, k, 1].astype(np.float64)
        x = x + dtf * vel * np.cos(yaw)
        y = y + dtf * vel * np.sin(yaw)
        yaw = yaw + dtf * vel * np.tan(s) / L
        vel = vel + dtf * a
        exp[:, k] = np.stack([x, y, yaw, vel], axis=-1)
    err = np.linalg.norm(got - exp) / np.linalg.norm(exp)
    print("CoreSim relnorm vs numpy ref:", err)
    for c in range(4):
        e = np.abs(got[:, :, c] - exp[:, :, c]).max()
        print(f"  lane {c}: absmax {e:.3e}")


# revision 12
# speedup vs baseline: 1.0379x; 1.0204x over previous
"""Trainium2 Bass kernel: K-step Euler rollout of a kinematic bicycle model.

Full inputs:
  initial_state [131072, 4] f32, controls [131072, 64, 2] f32,
  timestep scalar f32, agents_pars [131072, 2] f32
Output: [131072, 64, 4] f32 (state after each of the 64 steps).

Strategy: pure data parallel over 8 NeuronCores (16384 agents each).
Per core the sequential rollout is decomposed into 4 segmented prefix
scans (DVE tensor_tensor_scan with a 0/1 mask resetting state at agent
boundaries):
    V    = dt*vel   : V_k = V_{k-1} + dt^2*accel_k        (65-slot, slot0 = dt*vel0)
    yaw  : yaw_k = yaw_{k-1} + V_ex_k*tan(steer_k)/L      (65-slot, slot0 = yaw0)
    x    : x_k = x_{k-1} + V_ex_k*cos(yaw_ex_k)           (writes output lane)
    y    : y_k = y_{k-1} + V_ex_k*sin(yaw_ex_k)           (writes output lane)
Trig on ScalarE (Sin table; cos(yaw) via 1-2*sin^2(yaw/2) to stay in
[-pi,pi]); tensor products on GPSIMD; reciprocal on DVE.
"""
import os
import sys

for _p in ("/opt/trn_rl_repo", "/root/.axon_site/_ro/trn_rl_repo"):
    if os.path.isdir(_p) and _p not in sys.path:
        sys.path.insert(0, _p)

import numpy as np
import concourse.bass as bass
import concourse.bacc as bacc
import concourse.tile as tile
from concourse import mybir

F32 = mybir.dt.float32
AF = mybir.ActivationFunctionType
ALU = mybir.AluOpType

B = 131072
K = 64
NCORES = 8
BC = B // NCORES          # 16384 agents per core
P = 128                   # partitions
AG = 16                   # agents per partition per group
GRP = BC // (P * AG)      # 4 groups per core
PI = float(np.pi)

_cache = {}


FLAGS = set(os.environ.get("KFLAGS", "").split(","))


def _build(dt: float):
    """Build the per-core SPMD program (identical on all 8 cores)."""
    nc = bacc.Bacc("TRN2", debug=False)

    d_aux = nc.dram_tensor("aux", [BC, 6], F32, kind="ExternalInput").ap()
    d_ctrl = nc.dram_tensor("controls", [BC, K, 2], F32, kind="ExternalInput").ap()
    d_out = nc.dram_tensor("out", [BC, K, 4], F32, kind="ExternalOutput").ap()

    r_aux = d_aux.rearrange("(g p a) c -> g p (a c)", g=GRP, p=P, a=AG)
    r_ctrl = d_ctrl.rearrange("(g p a) k c -> g p (a k c)", g=GRP, p=P, a=AG)
    r_out = d_out.rearrange("(g p a) k c -> g p (a k c)", g=GRP, p=P, a=AG)

    flat = lambda t: t.rearrange("p a k -> p (a k)")

    with tile.TileContext(nc) as tc:
        with (
            tc.tile_pool(name="consts", bufs=1) as consts,
            tc.tile_pool(name="io", bufs=2) as io,
            tc.tile_pool(name="mid", bufs=1) as mid,
        ):
            # constants
            BF16 = mybir.dt.bfloat16
            mask65 = consts.tile([P, AG, 65], BF16)
            nc.vector.memset(mask65, 1.0)
            nc.vector.memset(mask65[:, :, 0], 0.0)
            mask64 = consts.tile([P, AG, 64], BF16)
            nc.vector.memset(mask64, 1.0)
            nc.vector.memset(mask64[:, :, 0], 0.0)
            c_dt2 = consts.tile([P, 1], F32)
            nc.vector.memset(c_dt2, dt * dt)
            c_pi2 = consts.tile([P, 1], F32)
            nc.vector.memset(c_pi2, PI / 2)
            c_m1 = consts.tile([P, 1], F32)
            nc.vector.memset(c_m1, -1.0)
            c_dt = consts.tile([P, 1], F32)
            nc.vector.memset(c_dt, dt)
            c_invdt = consts.tile([P, 1], F32)
            nc.vector.memset(c_invdt, 1.0 / dt)

            fronts = {}

            def front(g):
                ctrl_t = io.tile([P, AG, K, 2], F32, tag="ctrl", bufs=3, name=f"ctrl{g}")
                aux_t = io.tile([P, AG, 6], F32, tag="aux", bufs=3, name=f"aux{g}")
                nc.scalar.dma_start(ctrl_t, r_ctrl[g])
                nc.scalar.dma_start(aux_t, r_aux[g])
                accel = ctrl_t[:, :, :, 0]
                steer = ctrl_t[:, :, :, 1]
                vel0 = aux_t[:, :, 3]

                invL = mid.tile([P, AG], F32, tag="invL", bufs=3, name=f"invL{g}")
                nc.vector.reciprocal(invL, aux_t[:, :, 4])

                # velocity chain (V = dt*vel), 65-slot, scan in place
                t165 = mid.tile([P, AG, 65], F32, tag="t165", bufs=3, name=f"t165_{g}")
                nc.scalar.activation(t165[:, :, 1:65], accel, AF.Copy, scale=c_dt2)
                nc.scalar.activation(t165[:, :, 0], vel0, AF.Copy, scale=c_dt)
                nc.vector.tensor_tensor_scan(
                    flat(t165), flat(mask65), flat(t165), 0.0, ALU.mult, ALU.add
                )

                # steering: tan/L = sin * recip(cos) * invL
                sinS = mid.tile([P, AG, K], F32, tag="sinS", bufs=3, name=f"sinS{g}")
                nc.scalar.activation(sinS, steer, AF.Sin)
                cosS = mid.tile([P, AG, K], F32, tag="cosS", bufs=3, name=f"cosS{g}")
                nc.scalar.activation(cosS, steer, AF.Sin, bias=c_pi2)
                recipC = mid.tile([P, AG, K], F32, tag="recipC", bufs=2, name=f"recipC{g}")
                nc.vector.reciprocal_approx_fast(flat(recipC), flat(cosS))
                nc.gpsimd.tensor_mul(sinS, sinS, recipC)
                tanL = mid.tile([P, AG, K], F32, tag="tanL", bufs=3, name=f"tanL{g}")
                nc.gpsimd.tensor_mul(
                    tanL, sinS, invL.unsqueeze(2).broadcast_to([P, AG, K])
                )
                fronts[g] = (aux_t, t165, tanL)

            def back(g):
                aux_t, V65, tanL = fronts.pop(g)
                x0 = aux_t[:, :, 0]
                y0 = aux_t[:, :, 1]
                yaw0 = aux_t[:, :, 2]
                Vex = V65[:, :, 0:64]
                Vincl = V65[:, :, 1:65]
                out_t = io.tile([P, AG, K, 4], F32, tag="out", bufs=3, name=f"out{g}")

                # yaw chain, 65-slot, scan in place
                w265 = mid.tile([P, AG, 65], F32, tag="w265", bufs=3, name=f"w265_{g}")
                nc.gpsimd.tensor_mul(w265[:, :, 1:65], Vex, tanL)
                nc.scalar.activation(w265[:, :, 0], yaw0, AF.Copy)
                nc.vector.tensor_tensor_scan(
                    flat(w265), flat(mask65), flat(w265), 0.0, ALU.mult, ALU.add
                )
                Yex = w265[:, :, 0:64]
                Yincl = w265[:, :, 1:65]

                # trig of yaw: sin direct; cos = Sin(pi/2 - |yaw|)
                sinY = mid.tile([P, AG, K], F32, tag="sinY", bufs=3, name=f"sinY{g}")
                nc.scalar.activation(sinY, Yex, AF.Sin)
                yin = mid.tile([P, AG, K], F32, tag="yin", bufs=3, name=f"yin{g}")
                nc.gpsimd.tensor_mul(yin, Vex, sinY)
                nc.vector.tensor_add(yin[:, :, 0], yin[:, :, 0], y0)
                ylane = out_t[:, :, :, 1].rearrange("p a k -> p (a k)")
                nc.vector.tensor_tensor_scan(
                    ylane, flat(mask64), flat(yin), 0.0, ALU.mult, ALU.add
                )

                absY = mid.tile([P, AG, K], F32, tag="absY", bufs=3, name=f"absY{g}")
                nc.scalar.activation(absY, Yex, AF.Abs)
                cosY = mid.tile([P, AG, K], F32, tag="cosY", bufs=3, name=f"cosY{g}")
                nc.scalar.activation(cosY, absY, AF.Sin, scale=c_m1, bias=c_pi2)
                xin = mid.tile([P, AG, K], F32, tag="xin", bufs=3, name=f"xin{g}")
                nc.gpsimd.tensor_mul(xin, Vex, cosY)
                nc.vector.tensor_add(xin[:, :, 0], xin[:, :, 0], x0)
                xlane = out_t[:, :, :, 0].rearrange("p a k -> p (a k)")
                nc.vector.tensor_tensor_scan(
                    xlane, flat(mask64), flat(xin), 0.0, ALU.mult, ALU.add
                )

                # vel / yaw output lanes
                nc.scalar.activation(out_t[:, :, :, 3], Vincl, AF.Copy, scale=c_invdt)
                nc.vector.tensor_scalar(out_t[:, :, :, 2], Yincl, 1.0, None, ALU.mult)

                nc.sync.dma_start(r_out[g], out_t.rearrange("p a k c -> p (a k c)"))

            for g in range(GRP + 1):
                if g < GRP:
                    front(g)
                if g >= 1:
                    back(g - 1)

    nc.compile()
    return nc


def _get(dt: float):
    key = round(float(dt), 12)
    if key not in _cache:
        _cache[key] = _build(float(dt))
    return _cache[key]


def kernel(initial_state, controls, timestep, agents_pars, _trace=False):
    initial_state = np.ascontiguousarray(np.asarray(initial_state, dtype=np.float32))
    controls = np.ascontiguousarray(np.asarray(controls, dtype=np.float32))
    agents_pars = np.ascontiguousarray(np.asarray(agents_pars, dtype=np.float32))
    dt = float(np.asarray(timestep, dtype=np.float32))

    nc = _get(dt)
    aux = np.concatenate([initial_state, agents_pars], axis=1)
    in_maps = []
    for c in range(NCORES):
        s = slice(c * BC, (c + 1) * BC)
        in_maps.append({"aux": aux[s], "controls": controls[s]})
    from concourse import bass_utils

    r = bass_utils.run_bass_kernel_spmd(
        nc, in_maps, core_ids=list(range(NCORES)), trace=_trace
    )
    out = np.concatenate([r.results[c]["out"] for c in range(NCORES)], axis=0)
    if _trace:
        kernel.last_result = r
    return out


if __name__ == "__main__":
    # quick CoreSim check on one core's shard
    from concourse.bass_interp import CoreSim

    rng = np.random.default_rng(0)
    init = np.stack(
        [
            rng.normal(0, 10, BC),
            rng.normal(0, 10, BC),
            rng.normal(0, 0.5, BC),
            rng.normal(5, 2, BC),
        ],
        axis=-1,
    ).astype(np.float32)
    ctrl = (rng.standard_normal((BC, K, 2)) * np.array([1.0, 0.05])).astype(np.float32)
    pars = np.stack(
        [3 + 3 * rng.random(BC), 1.5 + rng.random(BC)], axis=-1
    ).astype(np.float32)
    dt = np.float32(0.1)

    nc = _get(float(dt))
    sim = CoreSim(nc, trace=False)
    sim.tensor("aux")[:] = np.concatenate([init, pars], axis=1)
    sim.tensor("controls")[:] = ctrl
    sim.simulate(check_with_hw=False)
    got = np.array(sim.tensor("out"))

    # numpy reference
    x, y, yaw, vel = (init[:, i].astype(np.float64) for i in range(4))
    L = pars[:, 0].astype(np.float64)
    exp = np.zeros((BC, K, 4))
    dtf = float(dt)
    for k in range(K):
        a = ctrl[:, k, 0].astype(np.float64)
        s = ctrl[:, k, 1].astype(np.float64)
        x = x + dtf * vel * np.cos(yaw)
        y = y + dtf * vel * np.sin(yaw)
        yaw = yaw + dtf * vel * np.tan(s) / L
        vel = vel + dtf * a
        exp[:, k] = np.stack([x, y, yaw, vel], axis=-1)
    err = np.linalg.norm(got - exp) / np.linalg.norm(exp)
    print("CoreSim relnorm vs numpy ref:", err)
    for c in range(4):
        e = np.abs(got[:, :, c] - exp[:, :, c]).max()
        print(f"  lane {c}: absmax {e:.3e}")


# revision 13
# speedup vs baseline: 1.0511x; 1.0127x over previous
"""Trainium2 Bass kernel: K-step Euler rollout of a kinematic bicycle model.

Full inputs:
  initial_state [131072, 4] f32, controls [131072, 64, 2] f32,
  timestep scalar f32, agents_pars [131072, 2] f32
Output: [131072, 64, 4] f32 (state after each of the 64 steps).

Strategy: pure data parallel over 8 NeuronCores (16384 agents each).
Per core the sequential rollout is decomposed into 4 segmented prefix
scans (DVE tensor_tensor_scan with a 0/1 mask resetting state at agent
boundaries):
    V    = dt*vel   : V_k = V_{k-1} + dt^2*accel_k        (65-slot, slot0 = dt*vel0)
    yaw  : yaw_k = yaw_{k-1} + V_ex_k*tan(steer_k)/L      (65-slot, slot0 = yaw0)
    x    : x_k = x_{k-1} + V_ex_k*cos(yaw_ex_k)           (writes output lane)
    y    : y_k = y_{k-1} + V_ex_k*sin(yaw_ex_k)           (writes output lane)
Trig on ScalarE (Sin table; cos(yaw) via 1-2*sin^2(yaw/2) to stay in
[-pi,pi]); tensor products on GPSIMD; reciprocal on DVE.
"""
import os
import sys

for _p in ("/opt/trn_rl_repo", "/root/.axon_site/_ro/trn_rl_repo"):
    if os.path.isdir(_p) and _p not in sys.path:
        sys.path.insert(0, _p)

import numpy as np
import concourse.bass as bass
import concourse.bacc as bacc
import concourse.tile as tile
from concourse import mybir

F32 = mybir.dt.float32
AF = mybir.ActivationFunctionType
ALU = mybir.AluOpType

B = 131072
K = 64
NCORES = 8
BC = B // NCORES          # 16384 agents per core
P = 128                   # partitions
AG = 16                   # agents per partition per group
GRP = BC // (P * AG)      # 4 groups per core
PI = float(np.pi)

_cache = {}


FLAGS = set(os.environ.get("KFLAGS", "").split(","))


def _build(dt: float):
    """Build the per-core SPMD program (identical on all 8 cores)."""
    nc = bacc.Bacc("TRN2", debug=False)

    d_aux = nc.dram_tensor("aux", [BC, 6], F32, kind="ExternalInput").ap()
    d_ctrl = nc.dram_tensor("controls", [BC, K, 2], F32, kind="ExternalInput").ap()
    d_out = nc.dram_tensor("out", [BC, K, 4], F32, kind="ExternalOutput").ap()

    r_aux = d_aux.rearrange("(g p a) c -> g p (a c)", g=GRP, p=P, a=AG)
    r_ctrl = d_ctrl.rearrange("(g p a) k c -> g p (a k c)", g=GRP, p=P, a=AG)
    r_out = d_out.rearrange("(g p a) k c -> g p (a k c)", g=GRP, p=P, a=AG)

    flat = lambda t: t.rearrange("p a k -> p (a k)")

    with tile.TileContext(nc) as tc:
        with (
            tc.tile_pool(name="consts", bufs=1) as consts,
            tc.tile_pool(name="io", bufs=2) as io,
            tc.tile_pool(name="mid", bufs=1) as mid,
        ):
            # constants
            BF16 = mybir.dt.bfloat16
            mask65 = consts.tile([P, AG, 65], BF16)
            nc.vector.memset(mask65, 1.0)
            nc.vector.memset(mask65[:, :, 0], 0.0)
            mask64 = consts.tile([P, AG, 64], BF16)
            nc.vector.memset(mask64, 1.0)
            nc.vector.memset(mask64[:, :, 0], 0.0)
            c_dt2 = consts.tile([P, 1], F32)
            nc.vector.memset(c_dt2, dt * dt)
            c_pi2 = consts.tile([P, 1], F32)
            nc.vector.memset(c_pi2, PI / 2)
            c_m1 = consts.tile([P, 1], F32)
            nc.vector.memset(c_m1, -1.0)
            c_dt = consts.tile([P, 1], F32)
            nc.vector.memset(c_dt, dt)
            c_invdt = consts.tile([P, 1], F32)
            nc.vector.memset(c_invdt, 1.0 / dt)

            fronts = {}

            def front(g):
                ctrl_t = io.tile([P, AG, K, 2], F32, tag="ctrl", bufs=3, name=f"ctrl{g}")
                aux_t = io.tile([P, AG, 6], F32, tag="aux", bufs=3, name=f"aux{g}")
                nc.scalar.dma_start(ctrl_t, r_ctrl[g])
                nc.scalar.dma_start(aux_t, r_aux[g])
                accel = ctrl_t[:, :, :, 0]
                steer = ctrl_t[:, :, :, 1]
                vel0 = aux_t[:, :, 3]

                invL = mid.tile([P, AG], F32, tag="invL", bufs=3, name=f"invL{g}")
                nc.vector.reciprocal(invL, aux_t[:, :, 4])

                # velocity chain (V = dt*vel), 65-slot, scan in place
                t165 = mid.tile([P, AG, 65], F32, tag="t165", bufs=3, name=f"t165_{g}")
                nc.scalar.activation(t165[:, :, 1:65], accel, AF.Copy, scale=c_dt2)
                nc.scalar.activation(t165[:, :, 0], vel0, AF.Copy, scale=c_dt)
                nc.vector.tensor_tensor_scan(
                    flat(t165), flat(mask65), flat(t165), 0.0, ALU.mult, ALU.add
                )

                # steering: tan/L = sin * recip(cos) * invL
                sinS = mid.tile([P, AG, K], F32, tag="sinS", bufs=3, name=f"sinS{g}")
                nc.scalar.activation(sinS, steer, AF.Sin)
                cosS = mid.tile([P, AG, K], F32, tag="cosS", bufs=3, name=f"cosS{g}")
                nc.scalar.activation(cosS, steer, AF.Sin, bias=c_pi2)
                recipC = mid.tile([P, AG, K], F32, tag="recipC", bufs=2, name=f"recipC{g}")
                nc.vector.reciprocal_approx_fast(flat(recipC), flat(cosS))
                nc.gpsimd.tensor_mul(sinS, sinS, recipC)
                tanL = mid.tile([P, AG, K], F32, tag="tanL", bufs=3, name=f"tanL{g}")
                nc.gpsimd.tensor_mul(
                    tanL, sinS, invL.unsqueeze(2).broadcast_to([P, AG, K])
                )
                fronts[g] = (aux_t, t165, tanL)

            def byaw(g):
                aux_t, V65, tanL = fronts[g]
                yaw0 = aux_t[:, :, 2]
                Vex = V65[:, :, 0:64]

                # yaw chain, 65-slot, scan in place
                w265 = mid.tile([P, AG, 65], F32, tag="w265", bufs=3, name=f"w265_{g}")
                nc.gpsimd.tensor_mul(w265[:, :, 1:65], Vex, tanL)
                nc.scalar.activation(w265[:, :, 0], yaw0, AF.Copy)
                nc.vector.tensor_tensor_scan(
                    flat(w265), flat(mask65), flat(w265), 0.0, ALU.mult, ALU.add
                )
                Yex = w265[:, :, 0:64]

                # trig of yaw: sin direct; cos = Sin(pi/2 - |yaw|)
                sinY = mid.tile([P, AG, K], F32, tag="sinY", bufs=3, name=f"sinY{g}")
                nc.scalar.activation(sinY, Yex, AF.Sin)
                absY = mid.tile([P, AG, K], F32, tag="absY", bufs=3, name=f"absY{g}")
                nc.scalar.activation(absY, Yex, AF.Abs)
                cosY = mid.tile([P, AG, K], F32, tag="cosY", bufs=3, name=f"cosY{g}")
                nc.scalar.activation(cosY, absY, AF.Sin, scale=c_m1, bias=c_pi2)
                fronts[g] = (aux_t, V65, w265, sinY, cosY)

            def bxy(g):
                aux_t, V65, w265, sinY, cosY = fronts.pop(g)
                x0 = aux_t[:, :, 0]
                y0 = aux_t[:, :, 1]
                Vex = V65[:, :, 0:64]
                Vincl = V65[:, :, 1:65]
                Yincl = w265[:, :, 1:65]
                out_t = io.tile([P, AG, K, 4], F32, tag="out", bufs=3, name=f"out{g}")

                yin = mid.tile([P, AG, K], F32, tag="yin", bufs=3, name=f"yin{g}")
                nc.gpsimd.tensor_mul(yin, Vex, sinY)
                nc.vector.tensor_add(yin[:, :, 0], yin[:, :, 0], y0)
                ylane = out_t[:, :, :, 1].rearrange("p a k -> p (a k)")
                nc.vector.tensor_tensor_scan(
                    ylane, flat(mask64), flat(yin), 0.0, ALU.mult, ALU.add
                )

                xin = mid.tile([P, AG, K], F32, tag="xin", bufs=3, name=f"xin{g}")
                nc.gpsimd.tensor_mul(xin, Vex, cosY)
                nc.vector.tensor_add(xin[:, :, 0], xin[:, :, 0], x0)
                xlane = out_t[:, :, :, 0].rearrange("p a k -> p (a k)")
                nc.vector.tensor_tensor_scan(
                    xlane, flat(mask64), flat(xin), 0.0, ALU.mult, ALU.add
                )

                # vel / yaw output lanes
                nc.scalar.activation(out_t[:, :, :, 3], Vincl, AF.Copy, scale=c_invdt)
                nc.scalar.activation(out_t[:, :, :, 2], Yincl, AF.Copy)

                nc.sync.dma_start(r_out[g], out_t.rearrange("p a k c -> p (a k c)"))

            for g in range(GRP + 2):
                if g < GRP:
                    front(g)
                if g >= 1 and g - 1 < GRP:
                    byaw(g - 1)
                if g >= 2:
                    bxy(g - 2)

    nc.compile()
    return nc


def _get(dt: float):
    key = round(float(dt), 12)
    if key not in _cache:
        _cache[key] = _build(float(dt))
    return _cache[key]


def kernel(initial_state, controls, timestep, agents_pars, _trace=False):
    initial_state = np.ascontiguousarray(np.asarray(initial_state, dtype=np.float32))
    controls = np.ascontiguousarray(np.asarray(controls, dtype=np.float32))
    agents_pars = np.ascontiguousarray(np.asarray(agents_pars, dtype=np.float32))
    dt = float(np.asarray(timestep, dtype=np.float32))

    nc = _get(dt)
    aux = np.concatenate([initial_state, agents_pars], axis=1)
    in_maps = []
    for c in range(NCORES):
        s = slice(c * BC, (c + 1) * BC)
        in_maps.append({"aux": aux[s], "controls": controls[s]})
    from concourse import bass_utils

    r = bass_utils.run_bass_kernel_spmd(
        nc, in_maps, core_ids=list(range(NCORES)), trace=_trace
    )
    out = np.concatenate([r.results[c]["out"] for c in range(NCORES)], axis=0)
    if _trace:
        kernel.last_result = r
    return out


if __name__ == "__main__":
    # quick CoreSim check on one core's shard
    from concourse.bass_interp import CoreSim

    rng = np.random.default_rng(0)
    init = np.stack(
        [
            rng.normal(0, 10, BC),
            rng.normal(0, 10, BC),
            rng.normal(0, 0.5, BC),
            rng.normal(5, 2, BC),
        ],
        axis=-1,
    ).astype(np.float32)
    ctrl = (rng.standard_normal((BC, K, 2)) * np.array([1.0, 0.05])).astype(np.float32)
    pars = np.stack(
        [3 + 3 * rng.random(BC), 1.5 + rng.random(BC)], axis=-1
    ).astype(np.float32)
    dt = np.float32(0.1)

    nc = _get(float(dt))
    sim = CoreSim(nc, trace=False)
    sim.tensor("aux")[:] = np.concatenate([init, pars], axis=1)
    sim.tensor("controls")[:] = ctrl
    sim.simulate(check_with_hw=False)
    got = np.array(sim.tensor("out"))

    # numpy reference
    x, y, yaw, vel = (init[:, i].astype(np.float64) for i in range(4))
    L = pars[:, 0].astype(np.float64)
    exp = np.zeros((BC, K, 4))
    dtf = float(dt)
    for k in range(K):
        a = ctrl[:, k, 0].astype(np.float64)
        s = ctrl[:, k, 1].astype(np.float64)
        x = x + dtf * vel * np.cos(yaw)
        y = y + dtf * vel * np.sin(yaw)
        yaw = yaw + dtf * vel * np.tan(s) / L
        vel = vel + dtf * a
        exp[:, k] = np.stack([x, y, yaw, vel], axis=-1)
    err = np.linalg.norm(got - exp) / np.linalg.norm(exp)
    print("CoreSim relnorm vs numpy ref:", err)
    for c in range(4):
        e = np.abs(got[:, :, c] - exp[:, :, c]).max()
        print(f"  lane {c}: absmax {e:.3e}")
